# revision 17
# baseline (speedup 1.0000x reference)
"""Trainium2 Bass kernel for DCN cross-interaction (cosine-sim dual softmax).

Math (per batch b):
    Qn = l2norm(Q[b]) [512, 256], Dn = l2norm(D[b]) [4096, 256]
    L = Qn @ Dn.T                       [512, 4096], values in [-1, 1]
    A_Q = softmax(L, axis=1)  = exp(L) / R,  R_q = sum_c exp(L[q, c])
    A_D = softmax(L.T, axis=1) = exp(L.T) / S,  S_c = sum_q exp(L[q, c])
    A_Qv = mean_q A_Q   [4096]
    A_Dv = mean_c A_D   [512]

Because L is bounded in [-1, 1], softmax needs no max-subtraction pass.

Layout strategy (per NeuronCore, 2 batches each, batch-parallel over 8 cores):
  - load Q, D natural-layout; compute 1/max(||row||, eps) via DVE fused
    square+reduce; normalize rows in natural layout; PE-transpose both into
    [d(part), n(free)] operand layout.
  - L tiles [128q, 512c] via fp32r matmul; ACT exp PSUM->SBUF with accum_out
    giving R partial sums for free.
  - S_c and 512*A_Qv_c in one PE matvec (lhsT = [ones, 1/R]) over E.
  - A_Q = E * (1/R) per-partition scale (DVE), DMA out.
  - L^T tiles [128c, 512q] via a second fp32r matmul (cheaper than PE-
    transposing E); A_D = exp(L^T - ln S_c) directly via ACT bias AP.
  - A_Dv via ones-matvec over A_D chunks accumulated in PSUM.
"""

import numpy as np

B, NQ, NC, DD = 16, 512, 4096, 256
N_CORES = 8
NB = B // N_CORES  # batches per core
EPS = 1e-8

_CACHE = {}


def _build_nc(n_reps=1):
    import concourse.bass as bass
    import concourse.tile as tile
    from concourse import masks, mybir

    f32 = mybir.dt.float32
    f32r = mybir.dt.float32r
    AF = mybir.ActivationFunctionType
    ALU = mybir.AluOpType
    AX = mybir.AxisListType

    nc = bass.Bass()

    q_in = nc.dram_tensor("q", [NB, NQ, DD], f32, kind="ExternalInput")
    d_in = nc.dram_tensor("d", [NB, NC, DD], f32, kind="ExternalInput")
    aq_out = nc.dram_tensor("aq", [NB, NQ, NC], f32, kind="ExternalOutput")
    ad_out = nc.dram_tensor("ad", [NB, NC, NQ], f32, kind="ExternalOutput")
    aqv_out = nc.dram_tensor("aqv", [NB, NC], f32, kind="ExternalOutput")
    adv_out = nc.dram_tensor("adv", [NB, NQ], f32, kind="ExternalOutput")
    svec_dram = nc.dram_tensor("svec_scratch", [NB, NC], f32)

    NQT = NQ // 128   # 4 q partition-tiles
    NCT = NC // 128   # 32 c partition-tiles
    NKT = DD // 128   # 2 contraction tiles
    NJ = NC // 512    # 8 c chunks
    NG = NCT // 4     # 8 groups of 4 c-tiles for A_D staging

    with tile.TileContext(nc) as tc:
        with (
            tc.tile_pool(name="const", bufs=1) as const_pool,
            tc.tile_pool(name="qnat", bufs=2) as qnat_pool,
            tc.tile_pool(name="dnat", bufs=1) as dnat_pool,
            tc.tile_pool(name="qt", bufs=2) as qt_pool,
            tc.tile_pool(name="dt", bufs=1) as dt_pool,
            tc.tile_pool(name="ebuf", bufs=1) as e_pool,
            tc.tile_pool(name="adst", bufs=2) as adst_pool,
            tc.tile_pool(name="small", bufs=2) as small_pool,
            tc.tile_pool(name="scr", bufs=2) as scr_pool,
            tc.tile_pool(name="svaq", bufs=1) as svaq_pool,
            tc.tile_pool(name="ps_t", bufs=2, space="PSUM") as pst_pool,
            tc.tile_pool(name="ps_mm", bufs=3, space="PSUM") as psmm_pool,
            tc.tile_pool(name="ps_vec", bufs=2, space="PSUM") as psvec_pool,
            tc.tile_pool(name="ps_adv", bufs=1, space="PSUM") as psadv_pool,
        ):
            identity = const_pool.tile([128, 128], f32)
            masks.make_identity(nc, identity[:])
            ones_f32 = const_pool.tile([128, 1], f32)
            nc.vector.memset(ones_f32[:], 1.0)
            ones_col = const_pool.tile([128, 1], f32r)
            nc.vector.tensor_copy(ones_col[:], ones_f32[:])

            # Dummy PE op that absorbs the gpsimd-identity wait so later
            # matmuls stay within the 2-wait ISA limit.
            pe_warm = pst_pool.tile([128, 128], f32, tag="pst")
            nc.tensor.transpose(pe_warm[:], identity[:], identity[:])

            def pe_touch(*aps):
                # Dependency-only PE nop: absorbs semaphore waits on the
                # given tiles so the next matmul stays within the ISA's
                # 2-wait limit.
                nop = nc.tensor.nop(hint="dep").ins
                nop.ins = [nc.tensor.lower_ap(ap) for ap in aps]

            for b in [bb % NB for bb in range(NB * n_reps)]:
                # ---------------- Phase A: load + norms + transposes ------
                q_nat = qnat_pool.tile([128, NQT, DD], f32)
                nc.sync.dma_start(
                    q_nat[:], q_in[b].rearrange("(t p) c -> p t c", p=128)
                )
                d_nat = dnat_pool.tile([128, NCT, DD], f32)
                nc.sync.dma_start(
                    d_nat[:], d_in[b].rearrange("(t p) c -> p t c", p=128)
                )

                # Row sum-of-squares via bn_stats: per 256-wide group the HW
                # returns (count, mean, count*var) for even and odd element
                # halves; sumsq = cv_e + cv_o + 128*(m_e^2 + m_o^2).
                def row_sumsq(nat, nt, ssq):
                    bn = scr_pool.tile([128, nt, 6], f32, tag="bn", bufs=2)
                    for t in range(nt):
                        nc.vector.bn_stats(bn[:, t], nat[:, t])
                    me, cve = bn[:, :, 1], bn[:, :, 2]
                    mo, cvo = bn[:, :, 4], bn[:, :, 5]
                    t1 = scr_pool.tile([128, nt], f32, tag="bn_t1", bufs=2)
                    t2 = scr_pool.tile([128, nt], f32, tag="bn_t2", bufs=2)
                    nc.vector.tensor_mul(t1[:], me, me)
                    nc.vector.tensor_mul(t2[:], mo, mo)
                    nc.vector.tensor_add(t1[:], t1[:], t2[:])
                    nc.vector.tensor_scalar_mul(t1[:], t1[:], float(DD // 2))
                    nc.vector.tensor_add(t2[:], cve, cvo)
                    nc.vector.tensor_add(ssq[:], t1[:], t2[:])

                ssq_q = small_pool.tile([128, NQT], f32)
                row_sumsq(q_nat, NQT, ssq_q)
                ssq_d = small_pool.tile([128, NCT], f32)
                row_sumsq(d_nat, NCT, ssq_d)

                inv_q = small_pool.tile([128, NQT], f32)
                nc.scalar.sqrt(inv_q[:], ssq_q[:])
                nc.vector.tensor_scalar_max(inv_q[:], inv_q[:], EPS)
                nc.vector.reciprocal(inv_q[:], inv_q[:])
                inv_d = small_pool.tile([128, NCT], f32)
                nc.scalar.sqrt(inv_d[:], ssq_d[:])
                nc.vector.tensor_scalar_max(inv_d[:], inv_d[:], EPS)
                nc.vector.reciprocal(inv_d[:], inv_d[:])

                for t in range(NQT):
                    nc.vector.tensor_scalar_mul(
                        q_nat[:, t], q_nat[:, t], inv_q[:, t : t + 1]
                    )
                for t in range(NCT):
                    nc.vector.tensor_scalar_mul(
                        d_nat[:, t], d_nat[:, t], inv_d[:, t : t + 1]
                    )

                # Transpose normalized Q/D into [d, n] operand layout.
                qt_sb = qt_pool.tile([128, NKT, NQ], f32r)
                pe_touch(q_nat[:])
                for kt in range(NKT):
                    pst = pst_pool.tile([128, 512], f32, tag="pst")
                    for t in range(NQT):
                        nc.tensor.transpose(
                            pst[:, t * 128 : (t + 1) * 128],
                            q_nat[:, t, kt * 128 : (kt + 1) * 128],
                            identity[:],
                        )
                    nc.vector.tensor_copy(qt_sb[:, kt], pst[:])
                dt_sb = dt_pool.tile([128, NKT, NC], f32r)
                pe_touch(d_nat[:])
                for kt in range(NKT):
                    for g in range(NG):
                        pst = pst_pool.tile([128, 512], f32, tag="pst")
                        for u in range(4):
                            t = g * 4 + u
                            nc.tensor.transpose(
                                pst[:, u * 128 : (u + 1) * 128],
                                d_nat[:, t, kt * 128 : (kt + 1) * 128],
                                identity[:],
                            )
                        nc.vector.tensor_copy(
                            dt_sb[:, kt, g * 512 : (g + 1) * 512], pst[:]
                        )

                # ---------------- Phase B: L matmuls + exp + R ------------
                e_sb = e_pool.tile([128, NQT, NC], f32r)
                invr = small_pool.tile([128, NQT], f32)
                for qt in range(NQT):
                    racc = small_pool.tile([128, NJ], f32, tag="racc")
                    for j in range(NJ):
                        ps = psmm_pool.tile([128, 512], f32, tag="mm")
                        for kt in range(NKT):
                            nc.tensor.matmul(
                                ps[:],
                                qt_sb[:, kt, qt * 128 : (qt + 1) * 128],
                                dt_sb[:, kt, j * 512 : (j + 1) * 512],
                                start=(kt == 0),
                                stop=(kt == NKT - 1),
                            )
                        nc.scalar.activation(
                            e_sb[:, qt, j * 512 : (j + 1) * 512],
                            ps[:],
                            AF.Exp,
                            accum_out=racc[:, j : j + 1],
                        )
                    nc.vector.reduce_sum(
                        invr[:, qt : qt + 1], racc[:], axis=AX.X
                    )
                    nc.vector.reciprocal(
                        invr[:, qt : qt + 1], invr[:, qt : qt + 1]
                    )

                # ---------------- Phase C: S + A_Qv matvec, A_Q out -------
                # matvec lhsT columns: col0 = 1/R (-> row 0 = 512*A_Qv),
                # col1 = ones (-> row 1 = S). Row 0 placement keeps the DVE
                # scale op at base partition 0 (engine constraint).
                lhsv = small_pool.tile([128, NQT, 2], f32r)
                for qt in range(NQT):
                    nc.vector.tensor_copy(lhsv[:, qt, 0:1], invr[:, qt : qt + 1])
                    nc.vector.tensor_copy(lhsv[:, qt, 1:2], ones_col[:])

                svaq = svaq_pool.tile([2, NC], f32)
                for j in range(NJ):
                    psv = psvec_pool.tile([2, 512], f32, tag="vec")
                    for qt in range(NQT):
                        nc.tensor.matmul(
                            psv[:],
                            lhsv[:, qt],
                            e_sb[:, qt, j * 512 : (j + 1) * 512],
                            start=(qt == 0),
                            stop=(qt == NQT - 1),
                        )
                    nc.vector.tensor_copy(svaq[:, j * 512 : (j + 1) * 512], psv[:])

                # A_Q_vector = row 0 / 512
                nc.vector.tensor_scalar_mul(svaq[0:1, :], svaq[0:1, :], 1.0 / NQ)
                nc.sync.dma_start(aqv_out[b : b + 1, :], svaq[0:1, :])

                # S (row 1) -> [128, 32] layout via DRAM round-trip, then -ln(S)
                nc.sync.dma_start(svec_dram[b : b + 1, :], svaq[1:2, :])
                s_t = small_pool.tile([128, NCT], f32, tag="s_t")
                nc.sync.dma_start(
                    s_t[:], svec_dram[b].rearrange("(t p) -> p t", p=128)
                )
                neglns = small_pool.tile([128, NCT], f32, tag="neglns")
                nc.scalar.activation(neglns[:], s_t[:], AF.Ln)
                nc.vector.tensor_scalar_mul(neglns[:], neglns[:], -1.0)

                # A_Q = E * invr (in place), DMA out
                for qt in range(NQT):
                    nc.vector.tensor_scalar_mul(
                        e_sb[:, qt], e_sb[:, qt], invr[:, qt : qt + 1]
                    )
                    nc.sync.dma_start(
                        aq_out[b, qt * 128 : (qt + 1) * 128, :], e_sb[:, qt].bitcast(f32)
                    )

                # ---------------- Phase D: L^T matmuls + A_D + A_Dv -------
                ps_adv = psadv_pool.tile([1, 512], f32)
                for g in range(NG):
                    adst = adst_pool.tile([128, 4, 512], f32r)
                    for u in range(4):
                        ct = g * 4 + u
                        ps = psmm_pool.tile([128, 512], f32, tag="mm")
                        for kt in range(NKT):
                            nc.tensor.matmul(
                                ps[:],
                                dt_sb[:, kt, ct * 128 : (ct + 1) * 128],
                                qt_sb[:, kt],
                                start=(kt == 0),
                                stop=(kt == NKT - 1),
                            )
                        nc.scalar.activation(
                            adst[:, u],
                            ps[:],
                            AF.Exp,
                            bias=neglns[:, ct : ct + 1],
                        )
                        nc.tensor.matmul(
                            ps_adv[:],
                            ones_col[:],
                            adst[:, u],
                            start=(ct == 0),
                            stop=(ct == NCT - 1),
                        )
                    nc.sync.dma_start(
                        ad_out[b, g * 512 : (g + 1) * 512, :].rearrange(
                            "(t p) qq -> p t qq", p=128
                        ),
                        adst[:].bitcast(f32),
                    )
                adv_sb = small_pool.tile([1, NQ], f32, tag="adv")
                nc.vector.tensor_scalar_mul(adv_sb[:], ps_adv[:], 1.0 / NC)
                nc.sync.dma_start(adv_out[b : b + 1, :], adv_sb[:])

    _split_excess_waits(nc, mybir)
    return nc


def _split_excess_waits(nc, mybir):
    """Walrus wait-slot capacities are tiny (1 for 4-byte self-loading
    matmuls, 2 for most compute ops). Hoist excess semaphore waits into
    same-engine NOPs injected immediately before the offending
    instruction — a NOP on the same engine completing first makes the
    split semantically identical."""
    # Empirically, walrus wait-slot capacity is 1 for every instruction
    # struct we emit (matmul S3_LW, activation S3D3_AC, NoOp CTRL_NO,
    # DMA pseudo, Drain). Split everything down to 1 wait.
    CAP = 1
    nsplit = 0
    for f in nc.m.functions:
        for blk in f.blocks:
            insts = list(blk.instructions)
            out = []
            for inst in insts:
                si = inst.sync_info
                cap = CAP
                if si is None or len(si.on_wait) <= cap:
                    out.append(inst)
                    continue
                SI = type(si)
                waits = list(si.on_wait)
                excess = waits[:-cap] if cap > 0 else waits
                keep = waits[len(excess):]
                # chain NOPs, each carrying 1 wait (NoOp slot capacity)
                while excess:
                    take, excess = excess[:1], excess[1:]
                    nop = mybir.InstNoOp(
                        name=f"I-waitsplit-{nsplit}", ins=[], outs=[]
                    )
                    nsplit += 1
                    nop.engine = inst.engine
                    nop.sync_info = SI(on_wait=take, on_update=[])
                    out.append(nop)
                inst.sync_info = SI(on_wait=keep, on_update=list(si.on_update))
                out.append(inst)
            if len(out) != len(insts):
                blk.instructions = out
    return nsplit


def _get_nc():
    if "nc" not in _CACHE:
        _CACHE["nc"] = _build_nc()
    return _CACHE["nc"]


def kernel(question_representation, context_representation, trace=False):
    from concourse.bass_utils import run_bass_kernel_spmd

    Q = np.ascontiguousarray(question_representation, dtype=np.float32)
    D = np.ascontiguousarray(context_representation, dtype=np.float32)
    assert Q.shape == (B, NQ, DD) and D.shape == (B, NC, DD)

    nc = _get_nc()
    in_maps = [
        {"q": Q[c * NB : (c + 1) * NB], "d": D[c * NB : (c + 1) * NB]}
        for c in range(N_CORES)
    ]
    res = run_bass_kernel_spmd(nc, in_maps, core_ids=list(range(N_CORES)), trace=trace)
    _CACHE["last_results"] = res

    aq = np.concatenate([r["aq"] for r in res.results], axis=0)
    ad = np.concatenate([r["ad"] for r in res.results], axis=0)
    aqv = np.concatenate([r["aqv"] for r in res.results], axis=0)
    adv = np.concatenate([r["adv"] for r in res.results], axis=0)
    return aq, ad, aqv, adv
